# revision 10
# baseline (speedup 1.0000x reference)
"""Multi-head causal attention (B=2, S=2048, D=1024, H=16) on 8 trn2 NeuronCores.

Sharding: data-parallel over batch (2) x tensor-parallel over heads (4 groups of
4 heads).  Core c = 4*b + g handles batch b, heads [4g, 4g+4).  Each core
computes a partial output  ctx_g @ Wo_g.T  [2048, 1024]; the host sums the 4
partials per batch.

Within-core dataflow (all matmuls in float32r = fp32 inputs truncated to fp22,
full PE rate at moving-dim >= 256):
  qT,kT = W @ X.T            [dk, s] head-pair-packed tiles [128, 2048]
  v     = X @ Wv.T           natural [s, dv] tiles [128, 256]
  S     = qT.T @ kT          per 128-row query tile, causal-trimmed 512 chunks,
                             two heads row-packed via tile_position
  P     = exp(8*S - 8*m)     online running max, exp on ScalarE (accum_out
                             produces row sums); normalization 1/Z and the
                             stale-max rescale are folded into one gpsimd
                             tensor_scalar pass over P
  P_T   = PE transpose       128x128 blocks, grouped into 256-wide q supertiles
  ctxT  = v.T @ P_T          two heads col-packed, accumulated over key tiles
  out  += ctxT.T @ WoT       per s-tile, PSUM -> SBUF -> DRAM
"""

import numpy as np

B, S, D, H = 2, 2048, 1024, 16
DK = D // H          # 64
JC = 256             # per-core projection width (4 heads * 64)
NQT = S // 128       # 16 query tiles
NU = S // 256        # 8 query supertiles
_SCALE = float(DK) ** 0.5  # 8.0  (reference multiplies scores by sqrt(dk))
_MASKVAL = -1.0e30

_cached = {}


def _build_nc():
    from contextlib import ExitStack

    import concourse.mybir as mybir
    import concourse.tile as tile
    from concourse import bacc

    F32 = mybir.dt.float32
    F32R = mybir.dt.float32r
    BF16 = mybir.dt.bfloat16
    EXP = mybir.ActivationFunctionType.Exp
    AX = mybir.AxisListType.X

    nc = bacc.Bacc("TRN2", target_bir_lowering=False)

    xtq_d = nc.dram_tensor("xtq", [D, S], F32R, kind="ExternalInput")
    xtk_d = nc.dram_tensor("xtk", [D, S], F32R, kind="ExternalInput")
    xtv_d = nc.dram_tensor("xtv", [D, S], F32R, kind="ExternalInput")
    wqt_d = nc.dram_tensor("wqt", [D, JC], F32R, kind="ExternalInput")
    wkt_d = nc.dram_tensor("wkt", [D, JC], F32R, kind="ExternalInput")
    wvt_d = nc.dram_tensor("wvt", [D, JC], F32R, kind="ExternalInput")
    wot_d = nc.dram_tensor("wot", [JC, D], F32R, kind="ExternalInput")
    cmask_d = nc.dram_tensor("cmask", [128, 128], F32, kind="ExternalInput")
    ident_d = nc.dram_tensor("ident", [128, 128], BF16, kind="ExternalInput")
    out_d = nc.dram_tensor("out", [S, D], F32, kind="ExternalOutput")

    def r(ap):
        return ap.bitcast(F32R)

    with tile.TileContext(nc) as tc, ExitStack() as top:
        res = top.enter_context(tc.tile_pool(name="res", bufs=1))
        stats = top.enter_context(tc.tile_pool(name="stats", bufs=1))

        # ---- resident tiles -------------------------------------------------
        # weight layouts: w?_sb[p, 256*dt + j] = W?T[128*dt + p, j]
        wq_sb = res.tile([128, 8, JC], F32R, tag="wq")
        wk_sb = res.tile([128, 8, JC], F32R, tag="wk")
        wv_sb = res.tile([128, 8, JC], F32R, tag="wv")
        nc.sync.dma_start(wq_sb, wqt_d[:, :].rearrange("(t p) j -> p t j", p=128))
        nc.sync.dma_start(wk_sb, wkt_d[:, :].rearrange("(t p) j -> p t j", p=128))
        nc.sync.dma_start(wv_sb, wvt_d[:, :].rearrange("(t p) j -> p t j", p=128))
        wo_sb = []
        for p2 in range(2):
            t = res.tile([128, D], F32R, tag=f"wo{p2}", name=f"wo{p2}")
            nc.sync.dma_start(t, wot_d[128 * p2 : 128 * (p2 + 1), :])
            wo_sb.append(t)
        cmask = res.tile([128, 128], F32, tag="cmask")
        ident = res.tile([128, 128], BF16, tag="ident")
        nc.sync.dma_start(cmask, cmask_d[:, :])
        nc.sync.dma_start(ident, ident_d[:, :])

        # projected tensors (resident through attention)
        # qts/kts[pair][p, s]: partitions = two heads' dk (2*64), free = s
        qts = [res.tile([128, S], F32R, tag=f"qts{i}", name=f"qts{i}") for i in range(2)]
        kts = [res.tile([128, S], F32R, tag=f"kts{i}", name=f"kts{i}") for i in range(2)]
        # v natural: vu[t][p, j]: s-tile t, all four heads' dv on free axis
        vu = [res.tile([128, JC], BF16, tag=f"vu{i}", name=f"vu{i}") for i in range(NQT)]
        # ctxT[pair][p, s]: partitions = two heads' dv, free = s
        ctxts = [res.tile([128, S], F32R, tag=f"ctx{i}", name=f"ctx{i}") for i in range(2)]

        # ---- stage B: projections ------------------------------------------
        with ExitStack() as stage_b:
            xpool = stage_b.enter_context(tc.tile_pool(name="xt", bufs=1))
            pjp = stage_b.enter_context(tc.tile_pool(name="pj", bufs=1, space="PSUM"))
            CH = 256
            for ch in range(S // CH):
                sl = slice(ch * CH, (ch + 1) * CH)
                xq_c = xpool.tile([128, 8, CH], F32R, tag="xq", bufs=2, name="xq")
                xk_c = xpool.tile([128, 8, CH], F32R, tag="xk", bufs=2, name="xk")
                xv_c = xpool.tile([128, 8, CH], F32R, tag="xv", bufs=2, name="xv")
                nc.sync.dma_start(xq_c, xtq_d[:, sl].rearrange("(t p) s -> p t s", p=128))
                nc.sync.dma_start(xk_c, xtk_d[:, sl].rearrange("(t p) s -> p t s", p=128))
                nc.sync.dma_start(xv_c, xtv_d[:, sl].rearrange("(t p) s -> p t s", p=128))
                for (wsb, dst) in ((wq_sb, qts), (wk_sb, kts)):
                    xc = xq_c if dst is qts else xk_c
                    for jt in range(2):
                        ps = pjp.tile([128, CH], F32, tag="pj", bufs=4, name="psqk")
                        for dt in range(8):
                            nc.tensor.matmul(
                                ps,
                                wsb[:, dt, 128 * jt : 128 * (jt + 1)],
                                xc[:, dt, :],
                                start=(dt == 0),
                                stop=(dt == 7),
                            )
                        if (ch + jt) % 2 == 0:
                            nc.vector.tensor_copy(dst[jt][:, sl], ps)
                        else:
                            nc.scalar.copy(dst[jt][:, sl], ps)
                for st in range(2):
                    ps = pjp.tile([128, JC], F32, tag="pj", bufs=4, name="psv")
                    for dt in range(8):
                        nc.tensor.matmul(
                            ps,
                            xv_c[:, dt, st * 128 : (st + 1) * 128],
                            wv_sb[:, dt, :],
                            start=(dt == 0),
                            stop=(dt == 7),
                        )
                    if st % 2 == 0:
                        nc.vector.tensor_copy(vu[2 * ch + st], ps)
                    else:
                        nc.scalar.copy(vu[2 * ch + st], ps)

        # ---- stage C/D: attention + output projection ----------------------
        with ExitStack() as stage_c:
            ppool = stage_c.enter_context(tc.tile_pool(name="pp", bufs=1))
            ptp = stage_c.enter_context(tc.tile_pool(name="ptp", bufs=1))
            obp = stage_c.enter_context(tc.tile_pool(name="obp", bufs=1))
            pss_p = stage_c.enter_context(tc.tile_pool(name="pss", bufs=1, space="PSUM"))
            pst_p = stage_c.enter_context(tc.tile_pool(name="pst", bufs=1, space="PSUM"))
            psc_p = stage_c.enter_context(tc.tile_pool(name="psc", bufs=1, space="PSUM"))
            pso_p = stage_c.enter_context(tc.tile_pool(name="pso", bufs=1, space="PSUM"))

            ncopy = 0  # round-robin DVE/ACT for PSUM->SBUF copies

            for u in range(NU):
                for p in range(2):
                    ptiles = {}  # (h, sq) -> P tile
                    zfs = {}     # (h, sq) -> list of per-chunk scale tiles
                    for sq in range(2):
                        qi = 2 * u + sq
                        qsl = slice(qi * 128, (qi + 1) * 128)
                        nfull = qi // 4
                        chunks = [(j, 512) for j in range(nfull)]
                        chunks.append((nfull, 128 * (qi % 4) + 128))
                        nch = len(chunks)
                        for h in range(2):
                            pt = ppool.tile([128, S], BF16, tag="P", bufs=6, name=f"P{p}{sq}{h}")
                            ptiles[(h, sq)] = pt
                            hsl = slice(64 * h, 64 * (h + 1))
                            negm8 = []
                            zs = []
                            m_run = None
                            for (j, w) in chunks:
                                ksl = slice(512 * j, 512 * j + w)
                                ps = pss_p.tile([128, 512], F32, tag="pss", bufs=3,
                                                name=f"ps{p}{sq}{h}{j}")
                                nc.tensor.matmul(
                                    ps[:, 0:w],
                                    qts[p][hsl, qsl],
                                    kts[p][hsl, ksl],
                                    start=True,
                                    stop=True,
                                    tile_position=(64 * h, 0),
                                )
                                if j == nfull:  # diagonal chunk: mask last block
                                    nc.vector.tensor_add(
                                        ps[:, w - 128 : w], ps[:, w - 128 : w], cmask
                                    )
                                cm = stats.tile([128, 1], F32, tag="cm", bufs=24, name="cm")
                                nc.vector.reduce_max(out=cm, in_=ps[:, 0:w], axis=AX)
                                if m_run is None:
                                    m_run = cm
                                else:
                                    m2 = stats.tile([128, 1], F32, tag="mr", bufs=24, name="mr")
                                    nc.vector.tensor_max(m2, m_run, cm)
                                    m_run = m2
                                nm = stats.tile([128, 1], F32, tag="nm", bufs=24, name="nm")
                                nc.vector.tensor_scalar_mul(nm, m_run, -_SCALE)
                                negm8.append(nm)
                                z = stats.tile([128, 1], F32, tag="z", bufs=24, name="z")
                                nc.scalar.activation(
                                    out=pt[:, ksl], in_=ps[:, 0:w], func=EXP,
                                    bias=nm, scale=_SCALE, accum_out=z,
                                )
                                zs.append(z)
                            # combine chunk stats -> per-chunk P scale factors
                            if nch == 1:
                                rz = stats.tile([128, 1], F32, tag="rz", bufs=24, name="rz")
                                nc.vector.reciprocal(rz, zs[0])
                                zf = [rz]
                            else:
                                fs = []
                                acc = zs[-1]
                                for j in range(nch - 1):
                                    f = stats.tile([128, 1], F32, tag="f", bufs=24, name="f")
                                    nc.scalar.activation(
                                        out=f, in_=negm8[j], func=EXP,
                                        bias=negm8[-1], scale=-1.0,
                                    )
                                    fs.append(f)
                                    zj = stats.tile([128, 1], F32, tag="zj", bufs=24, name="zj")
                                    nc.vector.tensor_mul(zj, zs[j], f)
                                    a2 = stats.tile([128, 1], F32, tag="za", bufs=24, name="za")
                                    nc.vector.tensor_add(a2, acc, zj)
                                    acc = a2
                                rz = stats.tile([128, 1], F32, tag="rz", bufs=24, name="rz")
                                nc.vector.reciprocal(rz, acc)
                                zf = []
                                for j in range(nch - 1):
                                    zfj = stats.tile([128, 1], F32, tag="zf", bufs=24, name="zf")
                                    nc.vector.tensor_mul(zfj, rz, fs[j])
                                    zf.append(zfj)
                                zf.append(rz)
                            # normalize P in place on gpsimd (idle engine)
                            for (j, w) in chunks:
                                nc.gpsimd.tensor_scalar_mul(
                                    pt[:, 512 * j : 512 * j + w],
                                    pt[:, 512 * j : 512 * j + w],
                                    zf[j],
                                )
                            zfs[(h, sq)] = zf
                    # transposes + PV for this (pair, supertile)
                    psc = psc_p.tile([128, 256], F32, tag="psc", bufs=2, name=f"psc{p}{u}")
                    for t in range(2 * u + 2):
                        last = t == 2 * u + 1
                        tsl = slice(128 * t, 128 * (t + 1))
                        for h in range(2):
                            pstile = pst_p.tile([128, 256], BF16, tag="pst", bufs=2,
                                                name=f"pst{h}{t}")
                            if not last:
                                nc.tensor.transpose(
                                    pstile[:, 0:128], ptiles[(h, 0)][:, tsl], ident
                                )
                            nc.tensor.transpose(
                                pstile[:, 128:256], ptiles[(h, 1)][:, tsl], ident
                            )
                            ptsb = ptp.tile([128, 256], BF16, tag="pt", bufs=3,
                                            name=f"pt{h}{t}")
                            csl = slice(128, 256) if last else slice(0, 256)
                            if ncopy % 2 == 0:
                                nc.vector.tensor_copy(ptsb[:, csl], pstile[:, csl])
                            else:
                                nc.scalar.copy(ptsb[:, csl], pstile[:, csl])
                            ncopy += 1
                            nc.tensor.matmul(
                                psc[64 * h : 64 * (h + 1), csl],
                                vu[t][:, 64 * (2 * p + h) : 64 * (2 * p + h + 1)],
                                ptsb[:, csl],
                                start=(t == 0),
                                stop=last,
                                tile_position=(0, 64 * h),
                                skip_group_check=True,
                            )
                    usl = slice(256 * u, 256 * (u + 1))
                    if u % 2 == 0:
                        nc.vector.tensor_copy(ctxts[p][:, usl], psc)
                    else:
                        nc.scalar.copy(ctxts[p][:, usl], psc)
                # output projection for the two finished s-tiles
                for st in (2 * u, 2 * u + 1):
                    ssl = slice(128 * st, 128 * (st + 1))
                    for oc in range(2):
                        osl = slice(512 * oc, 512 * (oc + 1))
                        pso = pso_p.tile([128, 512], F32, tag="pso", bufs=1,
                                         name=f"pso{st}{oc}")
                        for p in range(2):
                            nc.tensor.matmul(
                                pso,
                                ctxts[p][:, ssl],
                                wo_sb[p][:, osl],
                                start=(p == 0),
                                stop=(p == 1),
                            )
                        osb = obp.tile([128, 512], F32, tag="ob", bufs=2,
                                       name=f"ob{st}{oc}")
                        if oc == 0:
                            nc.vector.tensor_copy(osb, pso)
                        else:
                            nc.scalar.copy(osb, pso)
                        nc.sync.dma_start(out_d[ssl, osl], osb)

    nc.compile()
    return nc


def _get_nc():
    if "nc" not in _cached:
        _cached["nc"] = _build_nc()
    return _cached["nc"]


def _fp22(a):
    """Truncate fp32 to fp22 (e8m13) as the PE's float32r datapath does."""
    a = np.ascontiguousarray(a, dtype=np.float32)
    a.view(np.uint32)[...] &= np.uint32(0xFFFFFC00)
    return a


def _host_inputs(query, key, value, Wq, Wk, Wv, Wo):
    """Build the 8 per-core input dicts (host-side transposes/slices)."""
    f32 = np.float32
    xt = {}
    for b in range(B):
        xt[("q", b)] = _fp22(query[b].T)
        xt[("k", b)] = _fp22(key[b].T)
        xt[("v", b)] = _fp22(value[b].T)
    import ml_dtypes

    cmask = np.where(
        np.arange(128)[None, :] <= np.arange(128)[:, None], 0.0, _MASKVAL
    ).astype(f32)
    ident = np.eye(128).astype(ml_dtypes.bfloat16)
    in_maps = []
    for c in range(8):
        b, g = c // 4, c % 4
        jsl = slice(JC * g, JC * (g + 1))
        in_maps.append(
            {
                "xtq": xt[("q", b)],
                "xtk": xt[("k", b)],
                "xtv": xt[("v", b)],
                "wqt": _fp22(Wq[jsl, :].T),
                "wkt": _fp22(Wk[jsl, :].T),
                "wvt": _fp22(Wv[jsl, :].T),
                "wot": _fp22(Wo[:, jsl].T),
                "cmask": cmask,
                "ident": ident,
            }
        )
    return in_maps


def _numpy_fallback(query, key, value, mask, Wq, Wk, Wv, Wo):
    """Exact (chunked) numpy path for non-causal masks."""
    out = np.empty((B, S, D), dtype=np.float32)
    q = (query @ Wq.T).reshape(B, S, H, DK).transpose(0, 2, 1, 3)
    k = (key @ Wk.T).reshape(B, S, H, DK).transpose(0, 2, 1, 3)
    v = (value @ Wv.T).reshape(B, S, H, DK).transpose(0, 2, 1, 3)
    for b in range(B):
        ctx = np.empty((H, S, DK), dtype=np.float32)
        mb = mask[b] == 0
        for h in range(H):
            s = (q[b, h] @ k[b, h].T) * _SCALE
            s[mb] = np.finfo(np.float32).min
            s -= s.max(axis=1, keepdims=True)
            np.exp(s, out=s)
            s /= s.sum(axis=1, keepdims=True)
            ctx[h] = s @ v[b, h]
        out[b] = ctx.transpose(1, 0, 2).reshape(S, D) @ Wo.T
    return out


def kernel(query, key, value, mask, Wq, Wk, Wv, Wo):
    query = np.asarray(query, dtype=np.float32)
    key = np.asarray(key, dtype=np.float32)
    value = np.asarray(value, dtype=np.float32)
    mask = np.asarray(mask)
    Wq, Wk, Wv, Wo = (np.asarray(w, dtype=np.float32) for w in (Wq, Wk, Wv, Wo))

    tril = np.tril(np.ones((S, S), dtype=mask.dtype))
    if not all(np.array_equal(mask[b], tril) for b in range(B)):
        return _numpy_fallback(query, key, value, mask, Wq, Wk, Wv, Wo)

    from concourse.bass_utils import run_bass_kernel_spmd

    nc = _get_nc()
    in_maps = _host_inputs(query, key, value, Wq, Wk, Wv, Wo)
    res = run_bass_kernel_spmd(nc, in_maps, core_ids=list(range(8)))
    outs = [r["out"] for r in res.results]
    full = np.empty((B, S, D), dtype=np.float32)
    for b in range(B):
        full[b] = outs[4 * b] + outs[4 * b + 1] + outs[4 * b + 2] + outs[4 * b + 3]
    return full
